# revision 5
# baseline (speedup 1.0000x reference)
"""Bass/Trainium2 kernel for nn_GaussianNoise: out = noised + 0.1 * noise.

Full inputs (64,3,512,512) f32 are sharded batch-wise across 8 NeuronCores
(8 batches/core = 24 MiB per tensor per core). Pure memory-bound elementwise:
per core we stream 48 MiB in + 24 MiB out through SBUF.

Raw Bass (no Tile): this walrus build allows at most ONE instruction-embedded
sync wait, so all synchronization is done with sequencer-level wait_ge
commands instead. Three engine programs:
  SP  (nc.sync HWDGE ring):  load both input tiles in one 4 MiB DMA
  DVE:                        noise *= 0.1 ; noised += noise  (in place)
  ACT (nc.scalar HWDGE ring): store result tile (2 MiB)
K=4 SBUF slot ring gives load/compute/store overlap; loads and stores live
on different HWDGE rings so read and write streams share the 16 SDMA engines.
"""

import numpy as np

import concourse.bass as bass
from concourse import mybir
from concourse.bass_utils import run_bass_kernel_spmd

N_CORES = 8
B, C, H, W = 64, 3, 512, 512
PER_CORE_B = B // N_CORES                      # 8 batches per core
ELEMS = PER_CORE_B * C * H * W                 # 6,291,456 f32 per tensor per core
P = 128                                        # SBUF partitions
F = 4096                                       # tile free dim -> 2 MiB per half
T = ELEMS // (P * F)                           # 12 tiles
ROWS = T * P                                   # 1536
K = 4                                          # SBUF slot ring depth
SCALE = 2.0 * 0.05
F2 = 2 * F

_compiled = {}


def _build():
    nc = bass.Bass("TRN2", debug=False, num_devices=N_CORES)
    xy = nc.dram_tensor("xy", [2 * ROWS, F], mybir.dt.float32, kind="ExternalInput")
    out = nc.dram_tensor("out", [ROWS, F], mybir.dt.float32, kind="ExternalOutput")

    import contextlib

    ctx = contextlib.ExitStack()
    # Per-slot DMA semaphores: a single cumulative sem cannot order
    # individual DMAs (the 16 SDMA engines skew across consecutive
    # transfers), but same-slot DMAs are serialized by the dataflow, so
    # per-slot counts are exact.
    load_sems = [ctx.enter_context(nc.semaphore(f"load_sem{i}")) for i in range(K)]
    store_sems = [ctx.enter_context(nc.semaphore(f"store_sem{i}")) for i in range(K)]
    add_sem = ctx.enter_context(nc.semaphore("add_sem"))
    slots = [
        ctx.enter_context(nc.sbuf_tensor(f"slot{i}", [P, F2], mybir.dt.float32))
        for i in range(K)
    ]

    def load_src(t):
        # [128, 2, F]: partition stride F, half stride ROWS*F, unit inner
        return bass.AP(xy, t * P * F, [[F, P], [ROWS * F, 2], [1, F]])

    def load_dst(s):
        return bass.AP(slots[s], 0, [[F2, P], [F, 2], [1, F]])

    def noised_half(s):
        return bass.AP(slots[s], 0, [[F2, P], [1, F]])

    def noise_half(s):
        return bass.AP(slots[s], F, [[F2, P], [1, F]])

    def store_dst(t):
        return bass.AP(out, t * P * F, [[F, P], [1, F]])

    with nc.Block() as block:

        @block.sync
        def _(sync):
            for t in range(T):
                s = t % K
                if t >= K:
                    # slot reuse: wait until the slot's previous store drained
                    # (store completion implies the adds/loads for it too)
                    sync.wait_ge(store_sems[s], 16 * (t // K))
                sync.dma_start(load_dst(s), load_src(t)).then_inc(load_sems[s], 16)

        @block.vector
        def _(vector):
            for t in range(T):
                s = t % K
                vector.wait_ge(load_sems[s], 16 * (t // K + 1))
                vector.tensor_scalar_mul(noise_half(s), noise_half(s), SCALE)
                vector.tensor_add(
                    noised_half(s), noised_half(s), noise_half(s)
                ).then_inc(add_sem, 1)

        @block.scalar
        def _(scalar):
            for t in range(T):
                s = t % K
                scalar.wait_ge(add_sem, t + 1)
                scalar.dma_start(store_dst(t), noised_half(s)).then_inc(
                    store_sems[s], 16
                )
            for s in range(K):
                scalar.wait_ge(store_sems[s], 16 * ((T + K - 1 - s) // K))

    ctx.close()
    return nc


def _get_nc():
    if "nc" not in _compiled:
        _compiled["nc"] = _build()
    return _compiled["nc"]


def kernel(noised: np.ndarray, noise: np.ndarray, _trace: bool = False, **_trace_kwargs):
    nc = _get_nc()
    xs = np.ascontiguousarray(noised, dtype=np.float32).reshape(N_CORES, ROWS, F)
    ys = np.ascontiguousarray(noise, dtype=np.float32).reshape(N_CORES, ROWS, F)
    in_maps = [
        {"xy": np.concatenate([xs[c], ys[c]], axis=0)} for c in range(N_CORES)
    ]
    res = run_bass_kernel_spmd(
        nc, in_maps, list(range(N_CORES)), trace=_trace, **_trace_kwargs
    )
    out = np.stack([res.results[c]["out"] for c in range(N_CORES)])
    out = out.reshape(B, C, H, W)
    if _trace:
        kernel.last_results = res
    return out
